# revision 5
# baseline (speedup 1.0000x reference)
"""Multi-head attention on 8 Trainium2 NeuronCores (Bass/Tile).

Sharding: batch B=4 x head-groups 2 -> 8 cores. Each core computes full
attention for 1 batch element and 8 of 16 heads, producing a partial
output projection (Wo row-sharded); host sums the two partials per batch.

Device dataflow (per core), everything in "transposed" orientation so the
contraction dim always sits on SBUF partitions. All matmul operands are
bf16 (fp32 PSUM accumulation).
  Head: inputs stream in per 512-col S-group on 2 DMA queues (weights
  first, then kT/qT/vT group slices); K/Q t0 projections and V s-tiles
  chase the groups so the PE runs dense from ~12us. A ~48-matmul warm
  chain on dummy data bridges the HAM clock gate until real work lands.
  qT/kT act tiles are persistent (t1-3 projection chains reuse them, no
  re-DMA); vT uses 2 recycled group tiles.
  QT = (Wq^T qT) [512, S] bf16; KT likewise. V natural [S, 512] via
  lhsT=vT s-tiles (bv added with a K=1 ones-row matmul), stored per head
  with an appended ones column: Vp [k-tile, head, 65] bf16.
  scores^T[k,q] = (K_h^T tile).T @ Q_h^T, N=512 (K=64 contraction; even/
  odd heads on partitions 0-63/64-127 -> concurrent PE row-tiles).
  exp split: ACT does sub0 exact (Exp, scale=0.125); DVE does sub1 via a
  Schraudolph approximate exp: y_i16 = s*C1 + C2 -> int16 -> bitcast
  bf16 (one tensor_scalar). End-to-end rel err ~5.4e-3 (tol 2e-2).
  PV: out'[65, q] += Vp_tile.T @ expS_tile accumulated over 16 k-tiles;
  row 64 is the softmax denominator (ones column trick).
  A^T = out'[0:64] * reciprocal(out'[64]) broadcast -> bf16 [512, S].
  O^T[m, s] = Wo_chunk.T @ A^T chunk accumulated over 4 chunks -> fp32.
  O-proj for q 0:1536 interleaves into stage-2 columns (2 matmuls/tick
  from tick 209, gated by the hp3 norms); q 1536:2048 is the tail.
Host: out[b] = (O^T_hg0 + O^T_hg1).T + bo.
"""

import sys

sys.path.insert(0, "/opt/trn_rl_repo")

import ml_dtypes
import numpy as np

import concourse.bacc as bacc
import concourse.mybir as mybir
from concourse import tile
from concourse.bass_utils import run_bass_kernel_spmd

F32 = mybir.dt.float32
BF16 = mybir.dt.bfloat16
I16 = mybir.dt.int16
AF = mybir.ActivationFunctionType
ALU = mybir.AluOpType
NP_BF16 = ml_dtypes.bfloat16

H, DK, DV, DM = 16, 64, 64, 1024
B, S = 4, 2048
HL = H // 2          # heads per core
NB = HL * DK         # 512: per-core projection width
NDM = DM // 128      # 8 contraction chunks
NT = NB // 128       # 4 row-tiles of QT/KT/AT
NKT = S // 128       # 16 k-tiles
SCALE = 1.0 / 8.0    # 1/sqrt(DK)
# Schraudolph approximate exp for the DVE half of the softmax:
# exp(s*SCALE) ~ bitcast_bf16(int16(s*SC_C1 + SC_C2))
SC_C1 = SCALE * 128.0 * 1.4426950408889634  # SCALE * 128 * log2(e)
SC_C2 = 128.0 * (127.0 - 0.0436)

_CACHED_NC = None

import os

DEBUG_DUMP = bool(os.environ.get("KERNEL_DEBUG_DUMP"))


def _build():
    nc = bacc.Bacc("TRN2", debug=False)

    qT = nc.dram_tensor("qT", [128, NDM, S], BF16, kind="ExternalInput")
    kT = nc.dram_tensor("kT", [128, NDM, S], BF16, kind="ExternalInput")
    vT = nc.dram_tensor("vT", [128, NDM, S], BF16, kind="ExternalInput")
    wq = nc.dram_tensor("wq", [128, NDM, NB], BF16, kind="ExternalInput")
    wk = nc.dram_tensor("wk", [128, NDM, NB], BF16, kind="ExternalInput")
    wv = nc.dram_tensor("wv", [128, NDM, NB], BF16, kind="ExternalInput")
    wo = nc.dram_tensor("wo", [128, NT, DM], BF16, kind="ExternalInput")
    bq = nc.dram_tensor("bq", [NB], F32, kind="ExternalInput")
    bk = nc.dram_tensor("bk", [NB], F32, kind="ExternalInput")
    bv = nc.dram_tensor("bv", [NB], BF16, kind="ExternalInput")
    ones = nc.dram_tensor("ones", [1, 128], BF16, kind="ExternalInput")
    outT = nc.dram_tensor("outT", [DM, S], BF16, kind="ExternalOutput")
    if DEBUG_DUMP:
        qt_dbg = nc.dram_tensor("qt_dbg", [128, NT, S], BF16, kind="ExternalOutput")
        kt_dbg = nc.dram_tensor("kt_dbg", [128, NT, S], BF16, kind="ExternalOutput")
        vp_dbg = nc.dram_tensor(
            "vp_dbg", [128, NKT, HL, DV + 1], BF16, kind="ExternalOutput"
        )
        at_dbg = nc.dram_tensor("at_dbg", [128, NT, S], BF16, kind="ExternalOutput")
        ex_dbg = nc.dram_tensor("ex_dbg", [128, 2, 512], BF16, kind="ExternalOutput")

    with tile.TileContext(nc) as tc:
        with tc.tile_pool(name="persist", bufs=1) as persist:
            QT = persist.tile([128, NT, S], BF16)
            KT = persist.tile([128, NT, S], BF16)
            Vp = persist.tile([128, NKT, HL, DV + 1], BF16)
            wo_sb = persist.tile([128, NT, DM], BF16)
            bq_sb = persist.tile([128, NT], F32)
            bk_sb = persist.tile([128, NT], F32)
            bv_sb = persist.tile([1, NB], BF16)
            ones_sb = persist.tile([1, 128], BF16)
            qa = persist.tile([128, NDM, S], BF16, name="qa")
            ka = persist.tile([128, NDM, S], BF16, name="ka")

            nc.vector.memset(Vp[:, :, :, DV : DV + 1], 1.0)

            wgt_pool = tc.alloc_tile_pool(name="wgt", bufs=1)
            va_pool = tc.alloc_tile_pool(name="va", bufs=2)

            # ---- Stage 0: DMA schedule ----
            # Two queues (sync + gpsimd), each carrying half the chunks of
            # every tensor, issued in dependency-priority order: weights,
            # then per 512-col group g: kT_g, qT_g, vT_g. wo + misc last.
            nc.sync.dma_start(bv_sb[:], bv.rearrange("(o n) -> o n", o=1))
            nc.sync.dma_start(ones_sb[:], ones[:])
            nc.sync.dma_start(bq_sb[:], bq.rearrange("(t p) -> p t", p=128))
            nc.sync.dma_start(bk_sb[:], bk.rearrange("(t p) -> p t", p=128))

            wts = {}
            for key, src_w in (("k", wk), ("q", wq), ("v", wv)):
                wt = wgt_pool.tile([128, NDM, NB], BF16, tag=f"w{key}", name=f"w{key}")
                nc.sync.dma_start(wt[:, 0:4, :], src_w[:, 0:4, :])
                nc.gpsimd.dma_start(wt[:, 4:8, :], src_w[:, 4:8, :])
                wts[key] = wt

            acts = {"q": qa, "k": ka}
            va_tiles = [
                va_pool.tile([128, NDM, 512], BF16, tag="va", name=f"va{g}")
                for g in range(4)
            ]
            for g in range(4):
                gs = slice(g * 512, (g + 1) * 512)
                for key, src in (("k", kT), ("q", qT)):
                    nc.sync.dma_start(acts[key][:, 0:4, gs], src[:, 0:4, gs])
                    nc.gpsimd.dma_start(acts[key][:, 4:8, gs], src[:, 4:8, gs])
                nc.sync.dma_start(va_tiles[g][:, 0:4, :], vT[:, 0:4, gs])
                nc.gpsimd.dma_start(va_tiles[g][:, 4:8, :], vT[:, 4:8, gs])
            nc.sync.dma_start(wo_sb[:], wo[:])

            biases = {"q": bq_sb, "k": bk_sb}
            dsts = {"q": QT, "k": KT}

            # ---- Stage 1: per-group K/Q t0 projections + V s-tiles ----
            with tc.tile_pool(name="ps_s1", bufs=4, space="PSUM") as ps_s1:
                # HAM warm chain: ~48 N=512 matmuls on dummy data fill the
                # first ~12us (DMA-only window) so the clock gate is at 8/8
                # when the first projection matmul issues.
                warm_src = persist.tile([64, 512], BF16, name="warmsrc")
                nc.vector.memset(warm_src[:], 0.0)
                wps = ps_s1.tile([128, 512], F32, tag="warm")
                for _ in range(48):
                    nc.tensor.matmul(
                        wps[:],
                        warm_src[:, 0:128],
                        warm_src[:],
                        start=True,
                        stop=True,
                    )

                for g in range(4):
                    gs = slice(g * 512, (g + 1) * 512)
                    for key in ("k", "q"):
                        ps = ps_s1.tile([128, 512], F32, tag="ps")
                        for c in range(NDM):
                            nc.tensor.matmul(
                                ps[:],
                                wts[key][:, c, 0:128],
                                acts[key][:, c, gs],
                                start=(c == 0),
                                stop=(c == NDM - 1),
                            )
                        nc.vector.tensor_scalar_add(
                            dsts[key][:, 0, gs], ps[:], biases[key][:, 0:1]
                        )
                    for st in range(4 * g, 4 * g + 4):
                        ls = slice((st % 4) * 128, (st % 4) * 128 + 128)
                        ps = ps_s1.tile([128, 512], F32, tag="ps")
                        for c in range(NDM):
                            nc.tensor.matmul(
                                ps[:],
                                va_tiles[g][:, c, ls],
                                wts["v"][:, c, :],
                                start=(c == 0),
                                stop=False,
                            )
                        nc.tensor.matmul(
                            ps[:], ones_sb[0:1, :], bv_sb[0:1, :], start=False, stop=True
                        )
                        nc.vector.tensor_copy(
                            Vp[:, st, :, 0:DV],
                            ps[:].rearrange("p (h d) -> p h d", h=HL),
                        )

            if DEBUG_DUMP:
                nc.sync.dma_start(vp_dbg[:], Vp[:])

            # ---- Stage 2: attention with interleaved t=1..3 projections ----
            with tc.tile_pool(name="att", bufs=1) as att_pool:
                AT = att_pool.tile([128, NT, S], BF16)
                with (
                    tc.tile_pool(name="expS", bufs=6) as exp_pool,
                    tc.tile_pool(name="rec", bufs=2) as rec_pool,
                    tc.tile_pool(name="ost2", bufs=3) as ost2_pool,
                    tc.tile_pool(name="ps_sc", bufs=2, space="PSUM") as ps_sc,
                    tc.tile_pool(name="ps_pv", bufs=4, space="PSUM") as ps_pv,
                ):
                    # Flat column stream over (hp, qb, kt) with the PV
                    # matmuls skewed one column behind scores/exp so the PE
                    # never stalls on the exp of the column it just produced.
                    pv_store = {}
                    proj_fns = {}
                    for hp in range(HL // 2):
                        chains = []
                        if hp < NT - 1:
                            tn = hp + 1
                            for key in ("q", "k"):
                                for sq in range(2):
                                    for half in range(2):
                                        chains.append((key, tn, sq, half))
                        chain_ps = [None]

                        def make_proj_tick(chains, chain_ps):
                            def proj_tick(tick):
                                ci, step = tick // 8, tick % 8
                                if ci >= len(chains):
                                    return
                                key, tn, sq, half = chains[ci]
                                if step == 0:
                                    chain_ps[0] = ps_pv.tile(
                                        [128, 512], F32, tag="pv", name="projps"
                                    )
                                cps = chain_ps[0]
                                qs = slice(
                                    sq * 1024 + half * 512,
                                    sq * 1024 + half * 512 + 512,
                                )
                                nc.tensor.matmul(
                                    cps[:],
                                    wts[key][:, step, tn * 128 : (tn + 1) * 128],
                                    acts[key][:, step, qs],
                                    start=(step == 0),
                                    stop=(step == NDM - 1),
                                )
                                if step == NDM - 1:
                                    # ACT applies the bias (gpsimd can't read
                                    # PSUM; DVE is loaded with the Schraudolph
                                    # half of the softmax)
                                    nc.scalar.add(
                                        dsts[key][:, tn, qs],
                                        cps[:],
                                        biases[key][:, tn : tn + 1],
                                    )

                            return proj_tick

                        proj_fns[hp] = make_proj_tick(chains, chain_ps)

                    cols = [
                        (hp, qb, kt)
                        for hp in range(HL // 2)
                        for qb in range(4)
                        for kt in range(NKT)
                    ]

                    def emit_pv(hp, qb, kt, ex):
                        pvs = pv_store[(hp, qb)]
                        for sub in range(2):
                            nc.tensor.matmul(
                                pvs[sub][0 : DV + 1, :],
                                Vp[:, kt, hp * 2 + sub, :],
                                ex[:, sub, :],
                                start=(kt == 0),
                                stop=(kt == NKT - 1),
                            )

                    def emit_norm(hp, qb):
                        t = hp
                        qsl = slice(qb * 512, (qb + 1) * 512)
                        pvs = pv_store.pop((hp, qb))
                        for sub in range(2):
                            psl = slice(sub * 64, sub * 64 + 64)
                            pvp = pvs[sub]
                            rec = rec_pool.tile([1, 512], F32, tag="r")
                            recb = rec_pool.tile([64, 512], F32, tag="rb")
                            dcp = rec_pool.tile([1, 512], F32, tag="d")
                            # custom-DVE ucode mishandles base_partition=64
                            # PSUM reads; stage through partition 0
                            nc.vector.tensor_copy(dcp[:], pvp[DV : DV + 1, :])
                            nc.vector.reciprocal_approx_fast(rec[:], dcp[:])
                            nc.gpsimd.partition_broadcast(recb[:], rec[:])
                            nc.vector.tensor_mul(
                                AT[psl, t, qsl], pvp[0:DV, :], recb[:]
                            )

                    # O-projection for q 0:1536 interleaved into the column
                    # stream: 24 chains of 4 matmuls, 2 matmuls per tick from
                    # tick 209. qslice s is gated by norm(hp3, qb_s) which
                    # fires at tick 208/224/240 -- the 2/tick cadence lands
                    # each qslice's first chain exactly after its norm.
                    oproj_jobs = [
                        (m, qs) for qs in range(3) for m in range(NDM)
                    ]
                    oproj_state = {"i": 0, "ps": None}

                    def oproj_tick():
                        i = oproj_state["i"]
                        ci, step = i // NT, i % NT
                        if ci >= len(oproj_jobs):
                            return
                        m, qs = oproj_jobs[ci]
                        hs = slice(qs * 512, qs * 512 + 512)
                        if step == 0:
                            oproj_state["ps"] = ps_pv.tile(
                                [128, 512], F32, tag="pv", name="ops"
                            )
                        ps = oproj_state["ps"]
                        nc.tensor.matmul(
                            ps[:],
                            wo_sb[:, step, m * 128 : (m + 1) * 128],
                            AT[:, step, hs],
                            start=(step == 0),
                            stop=(step == NT - 1),
                        )
                        if step == NT - 1:
                            ot = ost2_pool.tile([128, 512], BF16, tag="os")
                            nc.scalar.copy(ot[:], ps[:])
                            nc.sync.dma_start(
                                outT[m * 128 : (m + 1) * 128, hs], ot[:]
                            )
                        oproj_state["i"] = i + 1

                    prev = None
                    for hp, qb, kt in cols:
                        t = hp
                        qsl = slice(qb * 512, (qb + 1) * 512)
                        if (hp, qb) not in pv_store:
                            pv_store[(hp, qb)] = [
                                ps_pv.tile([128, 512], F32, tag="pv", name=f"pv{i}")
                                for i in range(2)
                            ]
                        scp = ps_sc.tile([128, 2, 512], F32, tag="sc")
                        for sub in range(2):
                            psl = slice(sub * 64, sub * 64 + 64)
                            nc.tensor.matmul(
                                scp[:, sub, :],
                                KT[psl, t, kt * 128 : (kt + 1) * 128],
                                QT[psl, t, qsl],
                                start=True,
                                stop=True,
                            )
                        ex = exp_pool.tile([128, 2, 512], BF16, tag="e")
                        # sub0: exact exp on ACT; sub1: Schraudolph on DVE
                        nc.scalar.activation(
                            ex[:, 0, :], scp[:, 0, :], AF.Exp, scale=SCALE
                        )
                        nc.vector.tensor_scalar(
                            ex[:, 1, :].bitcast(I16),
                            scp[:, 1, :],
                            SC_C1,
                            SC_C2,
                            ALU.mult,
                            ALU.add,
                        )
                        if DEBUG_DUMP and hp == 0 and qb == 0 and kt == 0:
                            nc.sync.dma_start(ex_dbg[:], ex[:])
                        if prev is not None:
                            phps, pqb, pkt, pex = prev
                            emit_pv(phps, pqb, pkt, pex)
                            if pkt == NKT - 1:
                                emit_norm(phps, pqb)
                        proj_fns[hp](qb * NKT + kt)
                        ci_flat = (hp * 4 + qb) * NKT + kt
                        if ci_flat >= 209:
                            oproj_tick()
                            oproj_tick()
                        prev = (hp, qb, kt, ex)
                    phps, pqb, pkt, pex = prev
                    emit_pv(phps, pqb, pkt, pex)
                    emit_norm(phps, pqb)
                    while oproj_state["i"] < len(oproj_jobs) * NT:
                        oproj_tick()
                if DEBUG_DUMP:
                    nc.sync.dma_start(at_dbg[:], AT[:])
                    nc.sync.dma_start(qt_dbg[:], QT[:])
                    nc.sync.dma_start(kt_dbg[:], KT[:])

                # ---- Stage 3: output projection tail (q 1536:2048) ----
                with (
                    tc.tile_pool(name="ostage", bufs=3) as ostage,
                    tc.tile_pool(name="ps_o", bufs=2, space="PSUM") as ps_o,
                ):
                    hs = slice(1536, 2048)
                    for m in range(NDM):
                        ps = ps_o.tile([128, 512], F32, tag="po")
                        for cc in range(NT):
                            nc.tensor.matmul(
                                ps[:],
                                wo_sb[:, cc, m * 128 : (m + 1) * 128],
                                AT[:, cc, hs],
                                start=(cc == 0),
                                stop=(cc == NT - 1),
                            )
                        ot = ostage.tile([128, 512], BF16, tag="o")
                        nc.vector.tensor_copy(ot[:], ps[:])
                        nc.sync.dma_start(outT[m * 128 : (m + 1) * 128, hs], ot[:])
            va_pool.release()
            wgt_pool.release()

    nc.compile()
    return nc


def get_nc():
    global _CACHED_NC
    if _CACHED_NC is None:
        _CACHED_NC = _build()
    return _CACHED_NC


def _bf(x):
    return np.ascontiguousarray(np.asarray(x, np.float32)).astype(NP_BF16)


def _tile_rows(x):
    # [R, C] -> [128, R//128, C] so each SBUF partition's data is contiguous
    r, c = x.shape
    return np.ascontiguousarray(x.reshape(r // 128, 128, c).transpose(1, 0, 2))


def make_in_maps(queries, keys, values, Wq, bq, Wk, bk, Wv, bv, Wo, bo):
    queries = np.asarray(queries, np.float32)
    keys = np.asarray(keys, np.float32)
    values = np.asarray(values, np.float32)
    Wq = np.asarray(Wq, np.float32)
    Wk = np.asarray(Wk, np.float32)
    Wv = np.asarray(Wv, np.float32)
    Wo = np.asarray(Wo, np.float32)
    bq = np.asarray(bq, np.float32)
    bk = np.asarray(bk, np.float32)
    bv = np.asarray(bv, np.float32)
    ones = np.ones((1, 128), NP_BF16)
    in_maps = []
    for core in range(8):
        b, hg = divmod(core, 2)
        sl = slice(hg * NB, (hg + 1) * NB)
        in_maps.append(
            {
                "qT": _tile_rows(_bf(queries[b].T)),
                "kT": _tile_rows(_bf(keys[b].T)),
                "vT": _tile_rows(_bf(values[b].T)),
                "wq": _tile_rows(_bf(Wq[:, sl])),
                "wk": _tile_rows(_bf(Wk[:, sl])),
                "wv": _tile_rows(_bf(Wv[:, sl])),
                "wo": _tile_rows(_bf(Wo[sl, :])),
                "bq": np.ascontiguousarray(bq[sl]),
                "bk": np.ascontiguousarray(bk[sl]),
                "bv": _bf(bv[sl]),
                "ones": ones,
            }
        )
    return in_maps


def assemble(results, bo):
    bo = np.asarray(bo, np.float32)
    out = np.empty((B, S, DM), np.float32)
    for b in range(B):
        acc = np.asarray(results[2 * b]["outT"], np.float32) + np.asarray(
            results[2 * b + 1]["outT"], np.float32
        )
        out[b] = acc.T + bo
    return out


def run(trace=False, **inputs):
    if trace:
        # NTFF profiling shim: this image's antenv lacks axon_hooks.
        import types

        try:
            from antenv import axon_hooks  # noqa: F401
        except ImportError:
            from trn_agent_boot.trn_boot import _ntff_profile_via_ctypes

            mod = types.ModuleType("antenv.axon_hooks")
            _hook = _ntff_profile_via_ctypes("/opt/axon/libaxon_pjrt.so")
            mod.get_axon_ntff_profile_hook = lambda: _hook
            sys.modules["antenv.axon_hooks"] = mod
    nc = get_nc()
    bo = inputs["bo"]
    in_maps = make_in_maps(**inputs)
    res = run_bass_kernel_spmd(nc, in_maps, list(range(8)), trace=trace)
    return assemble(res.results, bo), res


def kernel(**inputs):
    out, _ = run(trace=False, **inputs)
    return out
